# revision 11
# baseline (speedup 1.0000x reference)
"""Trainium2 Bass kernel for nn_BidPrefix: per-row cumprod + prefix-product gathers.

Computation (per row of [B, 514] input):
    probs = row[0:512]; mp = int(row[512]); bid = int(row[513])
    cp[k] = prod(probs[0:k]), cp[0] = 1                      (k in 0..512)
    survival_rate = cp[bid]
    rate_last     = cp[mp] - cp[mp+1]

Strategy: pure data-parallel over 8 NeuronCores (8192 rows each). Per core:
64 row-tiles of 128 rows; cumprod via one DVE tensor_tensor_scan per tile;
per-row gathers via GPSIMD indirect_copy (16 tiles batched per call, indices
offset by t*513 into a [128, 16*513] cp super-tile; each 16-partition group
gathers its own rows' indices, the wanted value lands on the diagonal
j == p%16 which a masked multiply + segmented reduce extracts).

The walrus build in this container supports only ONE sync-wait slot per
instruction, so after Tile scheduling we split excess waits onto single-wait
NoOps (engine instructions only), and the kernel is structured so that every
DMA's tile deps involve at most one engine.
"""

import sys

if "/opt/trn_rl_repo" not in sys.path:
    sys.path.insert(0, "/opt/trn_rl_repo")

from contextlib import ExitStack

import numpy as np

import concourse.bass as bass
import concourse.tile as tile
from concourse import mybir
from concourse.bass_utils import run_bass_kernel_spmd

B = 65536
S = 512
N_CORES = 8
R = B // N_CORES          # rows per core
P = 128                   # partitions
T_PER_G = 16              # row-tiles per super-group
N_TILES = R // P          # 64
N_G = N_TILES // T_PER_G  # 4 super-groups
CPW = S + 1               # 513 cp columns per tile

# const tensor layout (free dim):
#   [0:768)    M3   : extraction mask, (k,t,j) -> 1.0 if j == p%16
#   [768:800)  O2   : (t,c) -> t*513           (idx offsets for (mp, bid))
#   [800:816)  O1   : t -> t*513 + 1           (idx offsets for mp+1)
C_M3, C_O2, C_O1, C_W = 0, 768, 800, 816

_cached = {}


def _build_consts() -> np.ndarray:
    c = np.zeros((P, C_W), np.float32)
    m3 = c[:, C_M3:C_O2].reshape(P, 3, T_PER_G, 16)
    for p in range(P):
        m3[p, :, :, p % 16] = 1.0
    o2 = c[:, C_O2:C_O1].reshape(P, T_PER_G, 2)
    o2[:, :, :] = (np.arange(T_PER_G) * CPW)[None, :, None]
    c[:, C_O1:C_W] = (np.arange(T_PER_G) * CPW + 1)[None, :]
    return c


def _split_sync_waits(nc: bass.Bass, gate=None, max_waits: int = 1) -> bass.Bass:
    """This walrus build allows ONE sync-wait slot per instruction.

    Engine instructions: move excess waits onto single-wait NoOps inserted
    just before (same engine; sequencers execute in order).
    DMA instructions: absorb ALL waits into SP-engine NoOps whose last one
    bumps the `gate` semaphore; the DMA then waits only on gate >= k.
    """
    dma_types = (mybir.InstDMACopy, mybir.InstDMA, mybir.InstTensorLoad,
                 mybir.InstTensorSave, mybir.InstDmaTransposeAnt)
    gate_k = 0
    for f in nc.m.functions:
        for bb in f.blocks:
            insts = bb.instructions
            out = []
            changed = False
            for inst in insts:
                si = inst.sync_info
                if si is not None and si.on_wait and len(si.on_wait) > max_waits:
                    waits = list(si.on_wait)
                    if isinstance(inst, dma_types):
                        assert gate is not None, "multi-wait DMA needs gate sem"
                        gate_k += 1
                        for j, w in enumerate(waits):
                            upd = []
                            if j == len(waits) - 1:
                                upd = [mybir.SyncUpdate(
                                    sync_type="semaphore", id=gate.num,
                                    ant_name=gate.name, update_mode="sem-inc",
                                    update_value=1, update_reg=None)]
                            out.append(mybir.InstNoOp(
                                name=f"{inst.name}-dmagate-{j}", ins=[], outs=[],
                                engine=mybir.EngineType.SP,
                                sync_info=mybir.SyncInfo(on_wait=[w],
                                                         on_update=upd),
                            ))
                        inst.sync_info = mybir.SyncInfo(
                            on_wait=[mybir.SyncWait(
                                sync_type="semaphore", id=gate.num,
                                ant_name=gate.name, wait_mode="sem-ge-imm",
                                wait_value=gate_k, wait_reg=None)],
                            on_update=list(si.on_update or []))
                    else:
                        for j, w in enumerate(waits[:-max_waits]):
                            out.append(mybir.InstNoOp(
                                name=f"{inst.name}-prewait-{j}", ins=[], outs=[],
                                engine=inst.engine,
                                sync_info=mybir.SyncInfo(on_wait=[w],
                                                         on_update=[]),
                            ))
                        inst.sync_info = mybir.SyncInfo(
                            on_wait=waits[-max_waits:],
                            on_update=list(si.on_update or []))
                    changed = True
                out.append(inst)
            if changed:
                bb.instructions = out
    return nc


def _build_program() -> bass.Bass:
    nc = bass.Bass("TRN2", target_bir_lowering=False, debug=False,
                   num_devices=N_CORES)
    x_ap = nc.dram_tensor("x", [R, S + 2], mybir.dt.float32,
                          kind="ExternalInput").ap()
    c_ap = nc.dram_tensor("c", [P, C_W], mybir.dt.float32,
                          kind="ExternalInput").ap()
    out_ap = nc.dram_tensor("out", [R, 2], mybir.dt.float32,
                            kind="ExternalOutput").ap()
    f32 = mybir.dt.float32
    gate = nc.alloc_semaphore("dma_gate")

    with tile.TileContext(nc) as tc, ExitStack() as ctx:
        cpool = ctx.enter_context(tc.tile_pool(name="consts", bufs=1))
        inp = ctx.enter_context(tc.tile_pool(name="inp", bufs=4))
        cpp = ctx.enter_context(tc.tile_pool(name="cp", bufs=2))
        small = ctx.enter_context(tc.tile_pool(name="small", bufs=2))

        ct = cpool.tile([P, C_W], f32)
        nc.sync.dma_start(ct[:], c_ap[:])
        zeros = cpool.tile([P, S], f32)
        nc.vector.memset(zeros[:], 0.0)

        for g in range(N_G):
            cp_sup = cpp.tile([P, T_PER_G, CPW], f32, tag="cp_sup")
            nc.vector.memset(cp_sup[:, :, 0], 1.0)  # cp[:, t, 0] = 1 seed
            mpbid = small.tile([P, T_PER_G, 2], f32, tag="mpbid")

            for t in range(T_PER_G):
                r0 = (g * T_PER_G + t) * P
                xt = inp.tile([P, S + 2], f32, tag="xt")
                nc.sync.dma_start(xt[:], x_ap[r0:r0 + P, :])
                nc.vector.tensor_tensor_scan(
                    cp_sup[:, t, 1:CPW], xt[:, 0:S], zeros[:], 1.0,
                    mybir.AluOpType.mult, mybir.AluOpType.add)
                # (mp, bid) columns via their own tiny DMA: keeps xt's readers
                # DVE-only so recycled-slot input DMAs carry a single wait
                nc.sync.dma_start(mpbid[:, t, :], x_ap[r0:r0 + P, S:S + 2])

            # indices (u16): idxu[:, 0, t] = mp + t*513, idxu[:, 1, t] = bid + t*513,
            #                idxu[:, 2, t] = mp + t*513 + 1
            idxf = small.tile([P, 3, T_PER_G], f32, tag="idxf")
            o2 = ct[:, C_O2:C_O1].rearrange("p (t c) -> p t c", c=2)
            nc.gpsimd.tensor_tensor(
                out=idxf[:, 0:2, :].transpose([0, 2, 1]),
                in0=mpbid[:, :, :], in1=o2, op=mybir.AluOpType.add)
            nc.gpsimd.tensor_tensor(
                out=idxf[:, 2, :], in0=mpbid[:, :, 0],
                in1=ct[:, C_O1:C_W], op=mybir.AluOpType.add)
            idxu = small.tile([P, 3, T_PER_G], mybir.dt.uint16, tag="idxu")
            nc.gpsimd.tensor_copy(idxu[:], idxf[:])

            # gathers: gt[:, k, 16*t + j] = cp_sup[group(p), idx of (row j, tile t)]
            gt = small.tile([P, 3, T_PER_G * 16], f32, tag="gt")
            cp_flat = cp_sup[:, :, :].rearrange("p t w -> p (t w)")
            for k in range(3):
                nc.gpsimd.indirect_copy(
                    gt[:, k, :], cp_flat, idxu[:, k, :],
                    i_know_ap_gather_is_preferred=True)

            # extraction: res[:, k, t] = sum_j gt[:, k, 16t+j] * M3[:, k, t, j]
            gtm = small.tile([P, 3, T_PER_G * 16], f32, tag="gtm")
            nc.vector.tensor_tensor(out=gtm[:], in0=gt[:], in1=ct[:, C_M3:C_O2],
                                    op=mybir.AluOpType.mult)
            res = small.tile([P, 3, T_PER_G], f32, tag="res")
            nc.vector.tensor_reduce(
                res[:], gtm[:, :, :].rearrange("p k (t j) -> p k t j", j=16),
                mybir.AxisListType.X, mybir.AluOpType.add)

            # outputs: out[:, t, 0] = res[:, 1, t] (survival)
            #          out[:, t, 1] = res[:, 0, t] - res[:, 2, t] (rate_last)
            ot = small.tile([P, T_PER_G, 2], f32, tag="ot")
            nc.vector.tensor_copy(ot[:, :, 0], res[:, 1, :])
            nc.vector.tensor_tensor(out=ot[:, :, 1], in0=res[:, 0, :],
                                    in1=res[:, 2, :], op=mybir.AluOpType.subtract)
            for t in range(T_PER_G):
                r0 = (g * T_PER_G + t) * P
                nc.sync.dma_start(out_ap[r0:r0 + P, :], ot[:, t, :])

    nc.sync.sem_clear(gate)  # restore zero for repeat executions
    return _split_sync_waits(nc, gate)


def kernel(inputs: np.ndarray):
    x = np.ascontiguousarray(np.asarray(inputs, np.float32))
    assert x.shape == (B, S + 2), x.shape
    if "nc" not in _cached:
        _cached["nc"] = _build_program()
        _cached["c"] = _build_consts()
    nc, c = _cached["nc"], _cached["c"]
    in_maps = [
        {"x": x[i * R:(i + 1) * R], "c": c} for i in range(N_CORES)
    ]
    res = run_bass_kernel_spmd(nc, in_maps, list(range(N_CORES)))
    out = np.concatenate([np.asarray(res.results[i]["out"])
                          for i in range(N_CORES)], axis=0)
    survival = np.ascontiguousarray(out[:, 0:1])
    rate_last = np.ascontiguousarray(out[:, 1:2])
    return survival, rate_last


# revision 14
# speedup vs baseline: 1.0572x; 1.0572x over previous
"""Trainium2 Bass kernel for nn_BidPrefix: per-row cumprod + prefix-product gathers.

Computation (per row of [B, 514] input):
    probs = row[0:512]; mp = int(row[512]); bid = int(row[513])
    cp[k] = prod(probs[0:k]), cp[0] = 1                      (k in 0..512)
    survival_rate = cp[bid]
    rate_last     = cp[mp] - cp[mp+1]

Strategy: pure data-parallel over 8 NeuronCores (8192 rows each). Per core:
64 row-tiles of 128 rows; cumprod via one DVE tensor_tensor_scan per tile;
per-row gathers via GPSIMD indirect_copy (16 tiles batched per call, indices
offset by t*513 into a [128, 16*513] cp super-tile; each 16-partition group
gathers its own rows' indices, the wanted value lands on the diagonal
j == p%16 which a masked multiply + segmented reduce extracts).

The walrus build in this container supports only ONE sync-wait slot per
instruction, so after Tile scheduling we split excess waits onto single-wait
NoOps (engine instructions only), and the kernel is structured so that every
DMA's tile deps involve at most one engine.
"""

import sys

if "/opt/trn_rl_repo" not in sys.path:
    sys.path.insert(0, "/opt/trn_rl_repo")

from contextlib import ExitStack

import numpy as np

import concourse.bass as bass
import concourse.tile as tile
from concourse import mybir
from concourse.bass_utils import run_bass_kernel_spmd

B = 65536
S = 512
N_CORES = 8
R = B // N_CORES          # rows per core
P = 128                   # partitions
T_PER_G = 16              # row-tiles per super-group
N_TILES = R // P          # 64
N_G = N_TILES // T_PER_G  # 4 super-groups
CPW = S + 1               # 513 cp columns per tile

# const tensor layout (free dim):
#   [0:768)    M3   : extraction mask, (k,t,j) -> 1.0 if j == p%16
#   [768:800)  O2   : (t,c) -> t*513           (idx offsets for (mp, bid))
#   [800:816)  O1   : t -> t*513 + 1           (idx offsets for mp+1)
C_M3, C_O2, C_O1, C_W = 0, 768, 800, 816

_cached = {}


def _build_consts() -> np.ndarray:
    c = np.zeros((P, C_W), np.float32)
    m3 = c[:, C_M3:C_O2].reshape(P, 3, T_PER_G, 16)
    for p in range(P):
        m3[p, :, :, p % 16] = 1.0
    o2 = c[:, C_O2:C_O1].reshape(P, T_PER_G, 2)
    o2[:, :, :] = (np.arange(T_PER_G) * CPW)[None, :, None]
    c[:, C_O1:C_W] = (np.arange(T_PER_G) * CPW + 1)[None, :]
    return c


def _split_sync_waits(nc: bass.Bass, gate=None, max_waits: int = 1) -> bass.Bass:
    """This walrus build allows ONE sync-wait slot per instruction.

    Engine instructions: move excess waits onto single-wait NoOps inserted
    just before (same engine; sequencers execute in order).
    DMA instructions: absorb ALL waits into SP-engine NoOps whose last one
    bumps the `gate` semaphore; the DMA then waits only on gate >= k.
    """
    dma_types = (mybir.InstDMACopy, mybir.InstDMA, mybir.InstTensorLoad,
                 mybir.InstTensorSave, mybir.InstDmaTransposeAnt)
    gate_k = 0
    for f in nc.m.functions:
        for bb in f.blocks:
            insts = bb.instructions
            out = []
            changed = False
            for inst in insts:
                si = inst.sync_info
                if si is not None and si.on_wait and len(si.on_wait) > max_waits:
                    waits = list(si.on_wait)
                    if isinstance(inst, dma_types):
                        assert gate is not None, "multi-wait DMA needs gate sem"
                        gate_k += 1
                        for j, w in enumerate(waits):
                            upd = []
                            if j == len(waits) - 1:
                                upd = [mybir.SyncUpdate(
                                    sync_type="semaphore", id=gate.num,
                                    ant_name=gate.name, update_mode="sem-inc",
                                    update_value=1, update_reg=None)]
                            out.append(mybir.InstNoOp(
                                name=f"{inst.name}-dmagate-{j}", ins=[], outs=[],
                                engine=mybir.EngineType.SP,
                                sync_info=mybir.SyncInfo(on_wait=[w],
                                                         on_update=upd),
                            ))
                        inst.sync_info = mybir.SyncInfo(
                            on_wait=[mybir.SyncWait(
                                sync_type="semaphore", id=gate.num,
                                ant_name=gate.name, wait_mode="sem-ge-imm",
                                wait_value=gate_k, wait_reg=None)],
                            on_update=list(si.on_update or []))
                    else:
                        for j, w in enumerate(waits[:-max_waits]):
                            out.append(mybir.InstNoOp(
                                name=f"{inst.name}-prewait-{j}", ins=[], outs=[],
                                engine=inst.engine,
                                sync_info=mybir.SyncInfo(on_wait=[w],
                                                         on_update=[]),
                            ))
                        inst.sync_info = mybir.SyncInfo(
                            on_wait=waits[-max_waits:],
                            on_update=list(si.on_update or []))
                    changed = True
                out.append(inst)
            if changed:
                bb.instructions = out
    return nc


def _build_program() -> bass.Bass:
    nc = bass.Bass("TRN2", target_bir_lowering=False, debug=False,
                   num_devices=N_CORES)
    x_ap = nc.dram_tensor("x", [R, S + 2], mybir.dt.float32,
                          kind="ExternalInput").ap()
    c_ap = nc.dram_tensor("c", [P, C_W], mybir.dt.float32,
                          kind="ExternalInput").ap()
    out_ap = nc.dram_tensor("out", [R, 2], mybir.dt.float32,
                            kind="ExternalOutput").ap()
    f32 = mybir.dt.float32
    gate = nc.alloc_semaphore("dma_gate")

    with tile.TileContext(nc) as tc, ExitStack() as ctx:
        cpool = ctx.enter_context(tc.tile_pool(name="consts", bufs=1))
        inp = ctx.enter_context(tc.tile_pool(name="inp", bufs=2))
        cpp = ctx.enter_context(tc.tile_pool(name="cp", bufs=2))
        small = ctx.enter_context(tc.tile_pool(name="small", bufs=2))

        ct = cpool.tile([P, C_W], f32)
        nc.sync.dma_start(ct[:], c_ap[:])
        zeros = cpool.tile([P, S], f32)
        nc.vector.memset(zeros[:], 0.0)

        for g in range(N_G):
            rg = g * T_PER_G * P
            rows = T_PER_G * P
            cp_sup = cpp.tile([P, T_PER_G, CPW], f32, tag="cp_sup")
            nc.vector.memset(cp_sup[:, :, 0], 1.0)  # cp[:, t, 0] = 1 seed
            mpbid = small.tile([P, T_PER_G, 2], f32, tag="mpbid")

            # one merged input DMA per super-group: row t*128+p -> [p, t, :]
            xt_sup = inp.tile([P, T_PER_G, S + 2], f32, tag="xt")
            nc.sync.dma_start(
                xt_sup[:],
                x_ap[rg:rg + rows, :].rearrange("(t p) w -> p t w", p=P))
            # (mp, bid) columns again via their own DMA: keeps xt_sup's
            # readers DVE-only so its recycle DMA carries few waits
            nc.sync.dma_start(
                mpbid[:],
                x_ap[rg:rg + rows, S:S + 2].rearrange("(t p) w -> p t w", p=P))

            for t in range(T_PER_G):
                nc.vector.tensor_tensor_scan(
                    cp_sup[:, t, 1:CPW], xt_sup[:, t, 0:S], zeros[:], 1.0,
                    mybir.AluOpType.mult, mybir.AluOpType.add)

            # indices (u16): idxu[:, 0, t] = mp + t*513, idxu[:, 1, t] = bid + t*513,
            #                idxu[:, 2, t] = mp + t*513 + 1
            idxf = small.tile([P, 3, T_PER_G], f32, tag="idxf")
            o2 = ct[:, C_O2:C_O1].rearrange("p (t c) -> p t c", c=2)
            nc.gpsimd.tensor_tensor(
                out=idxf[:, 0:2, :].transpose([0, 2, 1]),
                in0=mpbid[:, :, :], in1=o2, op=mybir.AluOpType.add)
            nc.gpsimd.tensor_tensor(
                out=idxf[:, 2, :], in0=mpbid[:, :, 0],
                in1=ct[:, C_O1:C_W], op=mybir.AluOpType.add)
            idxu = small.tile([P, 3, T_PER_G], mybir.dt.uint16, tag="idxu")
            nc.gpsimd.tensor_copy(idxu[:], idxf[:])

            # gathers: gt[:, k, 16*t + j] = cp_sup[group(p), idx of (row j, tile t)]
            gt = small.tile([P, 3, T_PER_G * 16], f32, tag="gt")
            cp_flat = cp_sup[:, :, :].rearrange("p t w -> p (t w)")
            for k in range(3):
                nc.gpsimd.indirect_copy(
                    gt[:, k, :], cp_flat, idxu[:, k, :],
                    i_know_ap_gather_is_preferred=True)

            # extraction: res[:, k, t] = sum_j gt[:, k, 16t+j] * M3[:, k, t, j]
            gtm = small.tile([P, 3, T_PER_G * 16], f32, tag="gtm")
            nc.vector.tensor_tensor(out=gtm[:], in0=gt[:], in1=ct[:, C_M3:C_O2],
                                    op=mybir.AluOpType.mult)
            res = small.tile([P, 3, T_PER_G], f32, tag="res")
            nc.vector.tensor_reduce(
                res[:], gtm[:, :, :].rearrange("p k (t j) -> p k t j", j=16),
                mybir.AxisListType.X, mybir.AluOpType.add)

            # outputs: out[:, t, 0] = res[:, 1, t] (survival)
            #          out[:, t, 1] = res[:, 0, t] - res[:, 2, t] (rate_last)
            ot = small.tile([P, T_PER_G, 2], f32, tag="ot")
            nc.vector.tensor_copy(ot[:, :, 0], res[:, 1, :])
            nc.vector.tensor_tensor(out=ot[:, :, 1], in0=res[:, 0, :],
                                    in1=res[:, 2, :], op=mybir.AluOpType.subtract)
            nc.sync.dma_start(
                out_ap[rg:rg + rows, :].rearrange("(t p) c -> p t c", p=P),
                ot[:])

    nc.sync.sem_clear(gate)  # restore zero for repeat executions
    return _split_sync_waits(nc, gate)


def kernel(inputs: np.ndarray):
    x = np.ascontiguousarray(np.asarray(inputs, np.float32))
    assert x.shape == (B, S + 2), x.shape
    if "nc" not in _cached:
        _cached["nc"] = _build_program()
        _cached["c"] = _build_consts()
    nc, c = _cached["nc"], _cached["c"]
    in_maps = [
        {"x": x[i * R:(i + 1) * R], "c": c} for i in range(N_CORES)
    ]
    res = run_bass_kernel_spmd(nc, in_maps, list(range(N_CORES)))
    out = np.concatenate([np.asarray(res.results[i]["out"])
                          for i in range(N_CORES)], axis=0)
    survival = np.ascontiguousarray(out[:, 0:1])
    rate_last = np.ascontiguousarray(out[:, 1:2])
    return survival, rate_last


# revision 17
# speedup vs baseline: 1.0736x; 1.0155x over previous
"""Trainium2 Bass kernel for nn_BidPrefix: per-row cumprod + prefix-product gathers.

Computation (per row of [B, 514] input):
    probs = row[0:512]; mp = int(row[512]); bid = int(row[513])
    cp[k] = prod(probs[0:k]), cp[0] = 1                      (k in 0..512)
    survival_rate = cp[bid]
    rate_last     = cp[mp] - cp[mp+1]

Strategy: pure data-parallel over 8 NeuronCores (8192 rows each). Per core:
64 row-tiles of 128 rows; cumprod via one DVE tensor_tensor_scan per tile;
per-row gathers via GPSIMD indirect_copy (16 tiles batched per call, indices
offset by t*513 into a [128, 16*513] cp super-tile; each 16-partition group
gathers its own rows' indices, the wanted value lands on the diagonal
j == p%16 which a masked multiply + segmented reduce extracts).

The walrus build in this container supports only ONE sync-wait slot per
instruction, so after Tile scheduling we split excess waits onto single-wait
NoOps (engine instructions only), and the kernel is structured so that every
DMA's tile deps involve at most one engine.
"""

import sys

if "/opt/trn_rl_repo" not in sys.path:
    sys.path.insert(0, "/opt/trn_rl_repo")

from contextlib import ExitStack

import numpy as np

import concourse.bass as bass
import concourse.tile as tile
from concourse import mybir
from concourse.bass_utils import run_bass_kernel_spmd

B = 65536
S = 512
N_CORES = 8
R = B // N_CORES          # rows per core
P = 128                   # partitions
T_PER_G = 16              # row-tiles per super-group
N_TILES = R // P          # 64
N_G = N_TILES // T_PER_G  # 4 super-groups
CPW = S + 1               # 513 cp columns per tile

# const tensor layout (free dim):
#   [0:768)    M3   : extraction mask, (k,t,j) -> 1.0 if j == p%16
#   [768:800)  O2   : (t,c) -> t*513           (idx offsets for (mp, bid))
#   [800:816)  O1   : t -> t*513 + 1           (idx offsets for mp+1)
C_M3, C_O2, C_O1, C_W = 0, 768, 800, 816

_cached = {}


def _build_consts() -> np.ndarray:
    c = np.zeros((P, C_W), np.float32)
    m3 = c[:, C_M3:C_O2].reshape(P, 3, T_PER_G, 16)
    for p in range(P):
        m3[p, :, :, p % 16] = 1.0
    o2 = c[:, C_O2:C_O1].reshape(P, T_PER_G, 2)
    o2[:, :, :] = (np.arange(T_PER_G) * CPW)[None, :, None]
    c[:, C_O1:C_W] = (np.arange(T_PER_G) * CPW + 1)[None, :]
    return c


def _split_sync_waits(nc: bass.Bass, gate=None, max_waits: int = 1) -> bass.Bass:
    """This walrus build allows ONE sync-wait slot per instruction.

    Engine instructions: move excess waits onto single-wait NoOps inserted
    just before (same engine; sequencers execute in order).
    DMA instructions: absorb ALL waits into SP-engine NoOps whose last one
    bumps the `gate` semaphore; the DMA then waits only on gate >= k.
    """
    dma_types = (mybir.InstDMACopy, mybir.InstDMA, mybir.InstTensorLoad,
                 mybir.InstTensorSave, mybir.InstDmaTransposeAnt)
    gate_k = 0
    for f in nc.m.functions:
        for bb in f.blocks:
            insts = bb.instructions
            out = []
            changed = False
            for inst in insts:
                si = inst.sync_info
                if si is not None and si.on_wait and len(si.on_wait) > max_waits:
                    waits = list(si.on_wait)
                    if isinstance(inst, dma_types):
                        assert gate is not None, "multi-wait DMA needs gate sem"
                        gate_k += 1
                        for j, w in enumerate(waits):
                            upd = []
                            if j == len(waits) - 1:
                                upd = [mybir.SyncUpdate(
                                    sync_type="semaphore", id=gate.num,
                                    ant_name=gate.name, update_mode="sem-inc",
                                    update_value=1, update_reg=None)]
                            out.append(mybir.InstNoOp(
                                name=f"{inst.name}-dmagate-{j}", ins=[], outs=[],
                                engine=mybir.EngineType.SP,
                                sync_info=mybir.SyncInfo(on_wait=[w],
                                                         on_update=upd),
                            ))
                        inst.sync_info = mybir.SyncInfo(
                            on_wait=[mybir.SyncWait(
                                sync_type="semaphore", id=gate.num,
                                ant_name=gate.name, wait_mode="sem-ge-imm",
                                wait_value=gate_k, wait_reg=None)],
                            on_update=list(si.on_update or []))
                    else:
                        for j, w in enumerate(waits[:-max_waits]):
                            out.append(mybir.InstNoOp(
                                name=f"{inst.name}-prewait-{j}", ins=[], outs=[],
                                engine=inst.engine,
                                sync_info=mybir.SyncInfo(on_wait=[w],
                                                         on_update=[]),
                            ))
                        inst.sync_info = mybir.SyncInfo(
                            on_wait=waits[-max_waits:],
                            on_update=list(si.on_update or []))
                    changed = True
                out.append(inst)
            if changed:
                bb.instructions = out
    return nc


def _build_program() -> bass.Bass:
    nc = bass.Bass("TRN2", target_bir_lowering=False, debug=False,
                   num_devices=N_CORES)
    x_ap = nc.dram_tensor("x", [R, S + 2], mybir.dt.float32,
                          kind="ExternalInput").ap()
    c_ap = nc.dram_tensor("c", [P, C_W], mybir.dt.float32,
                          kind="ExternalInput").ap()
    out_ap = nc.dram_tensor("out", [R, 2], mybir.dt.float32,
                            kind="ExternalOutput").ap()
    f32 = mybir.dt.float32
    gate = nc.alloc_semaphore("dma_gate")

    # row-to-partition layout: row p*64 + j lives on partition p, tile j.
    # Each partition's 16 rows per super-group are CONTIGUOUS in DRAM
    # (32.9 KB descriptors instead of 2 KB -> full DMA efficiency).
    x_r = x_ap.rearrange("(p j) w -> p j w", p=P)
    o_r = out_ap.rearrange("(p j) c -> p j c", p=P)

    with tile.TileContext(nc) as tc, ExitStack() as ctx:
        cpool = ctx.enter_context(tc.tile_pool(name="consts", bufs=1))
        inp = ctx.enter_context(tc.tile_pool(name="inp", bufs=2))
        cpp = ctx.enter_context(tc.tile_pool(name="cp", bufs=2))
        small = ctx.enter_context(tc.tile_pool(name="small", bufs=2))

        ct = cpool.tile([P, C_W], f32)
        nc.sync.dma_start(ct[:], c_ap[:])
        zeros = cpool.tile([P, S], f32)
        nc.vector.memset(zeros[:], 0.0)

        for g in range(N_G):
            j0 = g * T_PER_G
            cp_sup = cpp.tile([P, T_PER_G, CPW], f32, tag="cp_sup")
            nc.gpsimd.memset(cp_sup[:, :, 0], 1.0)  # cp[:, t, 0] = 1 seed
            mpbid = small.tile([P, T_PER_G, 2], f32, tag="mpbid")

            # one merged input DMA per super-group
            xt_sup = inp.tile([P, T_PER_G, S + 2], f32, tag="xt")
            nc.sync.dma_start(xt_sup[:], x_r[:, j0:j0 + T_PER_G, :])
            # (mp, bid) columns again via their own DMA: keeps xt_sup's
            # readers DVE-only so its recycle DMA carries few waits
            nc.sync.dma_start(mpbid[:], x_r[:, j0:j0 + T_PER_G, S:S + 2])

            for t in range(T_PER_G):
                nc.vector.tensor_tensor_scan(
                    cp_sup[:, t, 1:CPW], xt_sup[:, t, 0:S], zeros[:], 1.0,
                    mybir.AluOpType.mult, mybir.AluOpType.add)

            # indices (u16): idxu[:, 0, t] = mp + t*513, idxu[:, 1, t] = bid + t*513,
            #                idxu[:, 2, t] = mp + t*513 + 1
            idxf = small.tile([P, 3, T_PER_G], f32, tag="idxf")
            o2 = ct[:, C_O2:C_O1].rearrange("p (t c) -> p t c", c=2)
            nc.gpsimd.tensor_tensor(
                out=idxf[:, 0:2, :].transpose([0, 2, 1]),
                in0=mpbid[:, :, :], in1=o2, op=mybir.AluOpType.add)
            nc.gpsimd.tensor_tensor(
                out=idxf[:, 2, :], in0=mpbid[:, :, 0],
                in1=ct[:, C_O1:C_W], op=mybir.AluOpType.add)
            idxu = small.tile([P, 3, T_PER_G], mybir.dt.uint16, tag="idxu")
            nc.gpsimd.tensor_copy(idxu[:], idxf[:])

            # one gather for all 3 index sets (per-call ucode dispatch is ~6us,
            # so batch): gt[:, (k,t,j)] = cp_sup[group(p), idx of (row j, t, k)]
            gt = small.tile([P, 3 * T_PER_G * 16], f32, tag="gt")
            cp_flat = cp_sup[:, :, :].rearrange("p t w -> p (t w)")
            nc.gpsimd.indirect_copy(
                gt[:], cp_flat, idxu[:, :, :].rearrange("p k t -> p (k t)"),
                i_know_ap_gather_is_preferred=True)

            # extraction: res[:, k, t] = sum_j gt[:, k, 16t+j] * M3[:, k, t, j]
            gtm = small.tile([P, 3 * T_PER_G * 16], f32, tag="gtm")
            nc.vector.tensor_tensor(out=gtm[:], in0=gt[:], in1=ct[:, C_M3:C_O2],
                                    op=mybir.AluOpType.mult)
            res = small.tile([P, 3, T_PER_G], f32, tag="res")
            nc.vector.tensor_reduce(
                res[:], gtm[:].rearrange("p (k t j) -> p k t j", j=16, t=T_PER_G),
                mybir.AxisListType.X, mybir.AluOpType.add)

            # outputs: out[:, t, 0] = res[:, 1, t] (survival)
            #          out[:, t, 1] = res[:, 0, t] - res[:, 2, t] (rate_last)
            ot = small.tile([P, T_PER_G, 2], f32, tag="ot")
            nc.scalar.copy(ot[:, :, 0], res[:, 1, :])
            nc.vector.tensor_tensor(out=ot[:, :, 1], in0=res[:, 0, :],
                                    in1=res[:, 2, :], op=mybir.AluOpType.subtract)
            nc.sync.dma_start(o_r[:, j0:j0 + T_PER_G, :], ot[:])

    nc.sync.sem_clear(gate)  # restore zero for repeat executions
    return _split_sync_waits(nc, gate)


def kernel(inputs: np.ndarray):
    x = np.ascontiguousarray(np.asarray(inputs, np.float32))
    assert x.shape == (B, S + 2), x.shape
    if "nc" not in _cached:
        _cached["nc"] = _build_program()
        _cached["c"] = _build_consts()
    nc, c = _cached["nc"], _cached["c"]
    in_maps = [
        {"x": x[i * R:(i + 1) * R], "c": c} for i in range(N_CORES)
    ]
    res = run_bass_kernel_spmd(nc, in_maps, list(range(N_CORES)))
    out = np.concatenate([np.asarray(res.results[i]["out"])
                          for i in range(N_CORES)], axis=0)
    survival = np.ascontiguousarray(out[:, 0:1])
    rate_last = np.ascontiguousarray(out[:, 1:2])
    return survival, rate_last
